# revision 13
# baseline (speedup 1.0000x reference)
"""Cross-view attention Trainium2 kernel.

Reference computation (per sample b):
    q = Wq @ x1 + bq            (D=64, N)      x1 = view1[b] as (C, N)
    k = Wk @ x2 + bk            (D, N)
    v = Wv @ x2 + bv            (C, N)
    S = q^T k                   (N, N)
    P = softmax(S, axis=-1)
    out = v @ P^T               (C, N)
    y = gamma * out + x1

Sharding: data-parallel over batch B=8 across the 8 NeuronCores (one
sample per core), no collectives.

Device algorithm (per core), measured-rate driven design:
  - Inputs arrive as f16 (host converts; f16 has 10-bit mantissa and
    |x| <= ~6 so no range issue) and stay RESIDENT in SBUF (64 KB/part
    for v1+v2), so phase 1 needs no window DMAs, no dtype-convert DVE
    passes, and the epilogue residual re-read is free.
  - q/k projections are column-packed: q occupies PE columns 0-63,
    k columns 64-127 of the same [128, nwin] PSUM tile (concurrent on
    the PE, ~605ns/pair vs 2x429ns serial). Biases ride the ACT copy
    (per-partition bias). Partition-half replicas via SBUF->SBUF DMA.
  - v projection: 4 K=128 f16 matmuls per m-tile; bias via DVE
    tensor_add against a pre-broadcast [128, C] bias tile fused with
    the PSUM->SBUF copy (the old K=1 bias matmuls cost 609ns each).
  - Attention transposed, logit chain in f16 (1 PE-cycle/row; logits
    are O(50) so f16 is safe), value chain bf16 (exp(S) spans e^+-50,
    needs bf16 range): S^T tiles (128 m-part, 512 n-free) = kT^T @ qT
    as K=64 matmuls row-packed two-at-a-time (tile_position) which run
    concurrently on the PE; exp on ScalarE (no max subtraction needed
    in bf16); PV accumulates out[c,n] += vT^T @ ex in PSUM over all m.
  - The softmax denominator l[n] = sum_m ex[m,n] runs on the Vector
    engine (running-sum tiles, f32r) + ONE K=128 ones-matmul per
    chunk, replacing 32 accl matmuls/chunk on the saturated PE.
  - Final: out = acc * (gamma/l) + v1, written f16 (host upcasts).
"""

import sys

if "/opt/trn_rl_repo" not in sys.path:
    sys.path.insert(0, "/opt/trn_rl_repo")

import numpy as np

B, C, H, W = 8, 512, 64, 64
D = C // 8            # 64
N = H * W             # 4096
CC = C // 128         # 4 chunks of the channel dim
NCORES = 8

_compiled = {}


def _build(n=N, repeat=1, nwin=512, drop=()):
    from contextlib import ExitStack

    import concourse.mybir as mybir
    import concourse.tile as tile
    from concourse import bacc

    dt = mybir.dt
    f32, f32r, bf16 = dt.float32, dt.float32r, dt.bfloat16
    f16 = dt.float16
    AF = mybir.ActivationFunctionType

    nwin = min(nwin, n)
    nch = n // nwin       # output n-chunks
    mt = n // 128         # m tiles (key/value rows per tile)

    nc = bacc.Bacc("TRN2", target_bir_lowering=False, debug=False)
    v1 = nc.dram_tensor("v1", [C, n], f16, kind="ExternalInput").ap()
    v2 = nc.dram_tensor("v2", [C, n], f16, kind="ExternalInput").ap()
    wqT = nc.dram_tensor("wqT", [C, D], f32, kind="ExternalInput").ap()
    wkT = nc.dram_tensor("wkT", [C, D], f32, kind="ExternalInput").ap()
    wvT = nc.dram_tensor("wvT", [C, C], f32, kind="ExternalInput").ap()
    bq = nc.dram_tensor("bq", [1, D], f32, kind="ExternalInput").ap()
    bk = nc.dram_tensor("bk", [1, D], f32, kind="ExternalInput").ap()
    bv = nc.dram_tensor("bv", [1, C], f32, kind="ExternalInput").ap()
    gam = nc.dram_tensor("gam", [1, 1], f32, kind="ExternalInput").ap()
    out = nc.dram_tensor("out", [C, n], f16, kind="ExternalOutput").ap()

    v1p = v1.rearrange("(cc p) n -> p cc n", p=128)
    v2p = v2.rearrange("(cc p) n -> p cc n", p=128)
    outp = out.rearrange("(cc p) n -> p cc n", p=128)

    with tile.TileContext(nc) as tc, ExitStack() as top:
        consts = top.enter_context(tc.tile_pool(name="consts", bufs=1))

        # ---- constants ----
        wq_s = consts.tile([128, CC, D], f16, tag="wq")
        wk_s = consts.tile([128, CC, D], f16, tag="wk")
        wv_s = consts.tile([128, CC, C], f16, tag="wv")
        bqc_s = consts.tile([D, 1], f32, tag="bqc")   # ACT bias column
        bkc_s = consts.tile([64, 1], f32, tag="bkc")
        bvb_s = consts.tile([128, C], f32, tag="bvb")  # bv broadcast to all parts
        gam_s = consts.tile([1, 1], f32, tag="gam")
        ones_l = consts.tile([128, 1], f32r, tag="ones_l")   # K=128,M=1 lhsT (l)
        ones_pr = consts.tile([1, 128], f32r, tag="ones_pr")  # K=1,M=128 lhsT (rb bcast)

        with ExitStack() as p0:
            wstp = p0.enter_context(tc.tile_pool(name="wst", bufs=1))
            ps0 = p0.enter_context(tc.tile_pool(name="ps0", bufs=1, space="PSUM"))
            stage_w = wstp.tile([128, CC, C], f32, tag="stage_w")
            nc.scalar.dma_start(stage_w[:, :, :D], wqT.rearrange("(cc p) d -> p cc d", p=128))
            nc.vector.tensor_copy(wq_s[:], stage_w[:, :, :D])
            nc.scalar.dma_start(stage_w[:, :, D : 2 * D], wkT.rearrange("(cc p) d -> p cc d", p=128))
            nc.vector.tensor_copy(wk_s[:], stage_w[:, :, D : 2 * D])
            nc.scalar.dma_start(stage_w[:], wvT.rearrange("(cc p) c -> p cc c", p=128))
            nc.vector.tensor_copy(wv_s[:], stage_w[:])

            nc.scalar.dma_start(bqc_s[:], bq.rearrange("o d -> d o"))
            nc.scalar.dma_start(bkc_s[:], bk.rearrange("o d -> d o"))
            nc.scalar.dma_start(gam_s[:], gam[:])

            ones_f32 = wstp.tile([128, 128], f32, tag="ones_f32")
            nc.vector.memset(ones_f32[:], 1.0)
            nc.vector.tensor_copy(ones_l[:], ones_f32[:, :1])
            nc.vector.tensor_copy(ones_pr[:], ones_f32[:1, :])

            # bv broadcast: [1,C] -> [128,C] via K=1 ones matmul
            stage_b = wstp.tile([1, C], f32, tag="stage_b")
            nc.scalar.dma_start(stage_b[:], bv[:])
            ones_c = wstp.tile([1, 128], f32, tag="ones_c")
            nc.vector.tensor_copy(ones_c[:], ones_f32[:1, :])
            bvb_ps = ps0.tile([128, C], f32, tag="bvb_ps")
            nc.tensor.matmul(bvb_ps[:], ones_c[:], stage_b[:], start=True, stop=True)
            nc.vector.tensor_copy(bvb_s[:], bvb_ps[:])

        per = top.enter_context(tc.tile_pool(name="persist", bufs=1))
        v2_s = per.tile([128, CC, n], f16, tag="v2s")
        # The attention chain runs in f32r: TRN2 streams f32r moving
        # operands ~1.4x faster than 16-bit (measured 298ns vs 429ns per
        # K=128 Nf=512 matmul; K=64 pairs 328ns vs 600ns), and matmul
        # requires both operands f32r.  qT/kT duplicated across both
        # partition halves for the row-packed (tile_position) S^T pairs.
        qT_s = per.tile([128, n], f32r, tag="qT")
        kT_s = per.tile([128, n], f32r, tag="kT")
        vT_s = per.tile([128, mt, C], f32r, tag="vT")

        def emit_input_dmas(vi):
            # v2 only (v1 is streamed in windows by the q-projection and
            # the epilogue); called mid-phase-2 as a prefetch so the next
            # iteration's vproj never waits on DMA and the PE never idles
            # long enough for the HAM clock gate to re-throttle it
            nc.sync.dma_start(v2_s[:, :1, :], v2p[:, :1, :])
            nc.gpsimd.dma_start(v2_s[:, 1:2, :], v2p[:, 1:2, :])
            nc.scalar.dma_start(v2_s[:, 2:3, :], v2p[:, 2:3, :])
            nc.sync.dma_start(v2_s[:, 3:, :], v2p[:, 3:, :])

        def emit_iter(rep, prefetch=None):
            with ExitStack() as rctx:
                # ================= phase 1: projections =================
                if "proj" in drop:
                    nc.vector.memset(qT_s[:], 0.01)
                    nc.vector.memset(kT_s[:], 0.01)
                    nc.vector.memset(vT_s[:], 0.01)
                with ExitStack() as p1:
                    nch1 = 0 if "proj" in drop else nch
                    psqk = p1.enter_context(
                        tc.tile_pool(name=f"psqk{rep}", bufs=2, space="PSUM")
                    )
                    psv = p1.enter_context(
                        tc.tile_pool(name=f"psv{rep}", bufs=2, space="PSUM")
                    )
                    v1wp = p1.enter_context(tc.tile_pool(name=f"v1w{rep}", bufs=3))

                    # v projection first: gated only on the v2 DMA, so the
                    # PE starts ~10us in; the v1 DMA hides behind it
                    for m in range(0 if "proj" in drop else mt):
                        miw = slice(m * 128, (m + 1) * 128)
                        pv = psv.tile([128, C], f32, tag="psv")
                        for cc in range(CC):
                            nc.tensor.matmul(
                                pv[:],
                                v2_s[:, cc, miw],
                                wv_s[:, cc, :],
                                start=(cc == 0),
                                stop=(cc == CC - 1),
                            )
                        nc.vector.tensor_add(vT_s[:, m, :], pv[:], bvb_s[:])

                    for j in range(nch1):
                        jw = slice(j * nwin, (j + 1) * nwin)
                        v1w = v1wp.tile([128, CC, nwin], f16, tag="v1w", name="v1w")
                        eng = (nc.sync, nc.gpsimd, nc.scalar)[j % 3]
                        eng.dma_start(v1w[:], v1p[:, :, jw])
                        # q in PE columns 0-63, k in columns 64-127:
                        # concurrent on disjoint PE column groups; the two
                        # accumulation groups touch disjoint partition
                        # halves of the bank, so interleaved start is safe
                        ps = psqk.tile([128, nwin], f32, tag="psqk")
                        for cc in range(CC):
                            nc.tensor.matmul(
                                ps[:64, :],
                                wq_s[:, cc, :],
                                v1w[:, cc, :],
                                start=(cc == 0),
                                stop=(cc == CC - 1),
                                tile_position=(0, 0),
                            )
                            nc.tensor.matmul(
                                ps[64:128, :],
                                wk_s[:, cc, :],
                                v2_s[:, cc, jw],
                                start=(cc == 0),
                                stop=(cc == CC - 1),
                                tile_position=(0, 64),
                                skip_group_check=True,
                            )
                        nc.scalar.activation(
                            qT_s[:64, jw], ps[:64, :], AF.Identity, bias=bqc_s[:]
                        )
                        nc.scalar.activation(
                            kT_s[64:128, jw], ps[64:128, :], AF.Identity, bias=bkc_s[:]
                        )
                        nc.sync.dma_start(qT_s[64:128, jw], qT_s[:64, jw])
                        nc.gpsimd.dma_start(kT_s[:64, jw], kT_s[64:128, jw])

                # ================= phase 2: attention =================
                with ExitStack() as p2:
                    psS = p2.enter_context(
                        tc.tile_pool(name=f"psS{rep}", bufs=3, space="PSUM")
                    )
                    psA = p2.enter_context(
                        tc.tile_pool(name=f"psA{rep}", bufs=1, space="PSUM")
                    )
                    psL = p2.enter_context(
                        tc.tile_pool(name=f"psL{rep}", bufs=1, space="PSUM")
                    )
                    expp = p2.enter_context(tc.tile_pool(name=f"expp{rep}", bufs=8))
                    parep = p2.enter_context(tc.tile_pool(name=f"parep{rep}", bufs=3))
                    sump = p2.enter_context(tc.tile_pool(name=f"sump{rep}", bufs=2))
                    smalls = p2.enter_context(tc.tile_pool(name=f"smalls{rep}", bufs=2))
                    rbp = p2.enter_context(tc.tile_pool(name=f"rbp{rep}", bufs=2))
                    v1cp = p2.enter_context(tc.tile_pool(name=f"v1cp{rep}", bufs=6))
                    outp_sb = p2.enter_context(tc.tile_pool(name=f"outp{rep}", bufs=3))

                    if "pv" in drop or "accl" in drop:
                        inip = p2.enter_context(tc.tile_pool(name=f"inip{rep}", bufs=1))
                        ini = inip.tile([128, nwin], f32, tag="ini")
                        nc.vector.memset(ini[:], 1.0)
                    if "exp" in drop:
                        exst = p2.enter_context(tc.tile_pool(name=f"exst{rep}", bufs=1))
                        ex_static = [
                            exst.tile([128, nwin], f32r, tag=f"exs{h}", name=f"exs{h}")
                            for h in (0, 1)
                        ]
                        nc.vector.memset(ex_static[0][:], 0.01)
                        nc.vector.memset(ex_static[1][:], 0.01)

                    def emit_epilogue(j, accs, accl):
                        # y = acc * (gamma/l) + view1
                        jw = slice(j * nwin, (j + 1) * nwin)
                        v1cs = []
                        for ct in range(CC):
                            v1c = v1cp.tile([128, nwin], f16, tag="v1c", name="v1c")
                            eng = (nc.sync, nc.gpsimd, nc.scalar)[ct % 3]
                            eng.dma_start(v1c[:], v1p[:, ct, jw])
                            v1cs.append(v1c)
                        l_sb = smalls.tile([1, nwin], f32, tag="l", name="l_sb")
                        nc.vector.tensor_copy(l_sb[:], accl[:])
                        r_sb = smalls.tile([1, nwin], f32, tag="r", name="r_sb")
                        nc.vector.reciprocal(r_sb[:], l_sb[:])
                        rg_sb = smalls.tile([1, nwin], f32r, tag="rg", name="rg_sb")
                        nc.scalar.activation(rg_sb[:], r_sb[:], AF.Copy, scale=gam_s[:])
                        rb_ps = psL.tile([128, nwin], f32, tag="accl", name="rb_ps")
                        nc.tensor.matmul(rb_ps[:], ones_pr[:], rg_sb[:], start=True, stop=True)
                        rb_sb = rbp.tile([128, nwin], f32, tag="rb", name="rb_sb")
                        nc.vector.tensor_copy(rb_sb[:], rb_ps[:])
                        for ct in range(CC):
                            t_sb = outp_sb.tile([128, nwin], f32, tag="t", name="t_sb")
                            nc.vector.tensor_mul(t_sb[:], accs[ct][:], rb_sb[:])
                            o_sb = outp_sb.tile([128, nwin], f16, tag="o", name="o_sb")
                            nc.vector.tensor_add(o_sb[:], t_sb[:], v1cs[ct][:])
                            nc.sync.dma_start(outp[:, ct, jw], o_sb[:])

                    npairs = mt // 2
                    pend_epi = None
                    for j in range(nch):
                        jw = slice(j * nwin, (j + 1) * nwin)
                        # one PSUM tile (= one full bank) per output c-chunk:
                        # accumulation groups must not share a bank
                        accs = [
                            psA.tile([128, nwin], f32, tag=f"acc{ct}", name=f"acc{ct}")
                            for ct in range(CC)
                        ]
                        if "pv" in drop:
                            for ct in range(CC):
                                nc.vector.tensor_copy(accs[ct][:], ini[:])
                        # software pipeline: issue S^T/exp of pair i+1 before
                        # the P.V matmuls of pair i, so ScalarE's exp overlaps
                        # TensorE's P.V; the previous chunk's epilogue is
                        # emitted after this chunk's first S^T pair
                        prev_exs = None
                        sumE = None
                        pares = []
                        for m2 in range(npairs + 1):
                            exs = []
                            if m2 < npairs:
                                sts = []
                                for half in (0, 1):
                                    m = 2 * m2 + half
                                    mw = slice(m * 128, (m + 1) * 128)
                                    hp = slice(64 * half, 64 * half + 64)
                                    if "st" in drop:
                                        sts.append(None)
                                        continue
                                    st = psS.tile([128, nwin], f32, tag="st", name="st")
                                    nc.tensor.matmul(
                                        st[:],
                                        kT_s[hp, mw],
                                        qT_s[hp, jw],
                                        start=True,
                                        stop=True,
                                        tile_position=(64 * half, 0),
                                    )
                                    sts.append(st)
                                if "exp" in drop:
                                    exs = ex_static
                                else:
                                    for half in (0, 1):
                                        ex = expp.tile(
                                            [128, nwin], f32r, tag="ex", name="ex"
                                        )
                                        nc.scalar.activation(
                                            ex[:], sts[half][:], AF.Exp
                                        )
                                        exs.append(ex)
                            if m2 > 0:
                                for half in (0, 1):
                                    m = 2 * (m2 - 1) + half
                                    ex = prev_exs[half]
                                    for ct in range(CC if "pv" not in drop else 0):
                                        nc.tensor.matmul(
                                            accs[ct][:],
                                            vT_s[:, m, ct * 128 : (ct + 1) * 128],
                                            ex[:],
                                            start=(m == 0),
                                            stop=(m == mt - 1),
                                        )
                                if "accl" not in drop:
                                    # pair-partial on DVE: frees the ex tiles
                                    # immediately (the serial denominator
                                    # chain below runs on these, so the exp
                                    # pool never backs up behind it)
                                    pare = parep.tile(
                                        [128, nwin], f32r, tag="pare", name="pare"
                                    )
                                    nc.vector.tensor_add(
                                        pare[:], prev_exs[0][:], prev_exs[1][:]
                                    )
                                    if sumE is None:
                                        sumE = pare
                                    else:
                                        sacc = sump.tile(
                                            [128, nwin], f32r, tag="sumE", name="sacc"
                                        )
                                        nc.vector.tensor_add(
                                            sacc[:], sumE[:], pare[:]
                                        )
                                        sumE = sacc
                            if m2 == 1 and pend_epi is not None:
                                # emitted AFTER this chunk's first PV block:
                                # the rb broadcast matmul inside depends on a
                                # DVE->ACT chain, and the in-order PE queue
                                # must have PV work ahead of it to hide that
                                emit_epilogue(*pend_epi)
                                pend_epi = None
                                if j == 1 and prefetch is not None:
                                    prefetch()
                            prev_exs = exs
                        accl = psL.tile([1, nwin], f32, tag="accl")
                        if "accl" not in drop:
                            nc.tensor.matmul(
                                accl[:], ones_l[:], sumE[:], start=True, stop=True
                            )
                        else:
                            nc.vector.tensor_copy(accl[:], ini[:1, :])
                        pend_epi = (j, accs, accl)
                    emit_epilogue(*pend_epi)

        if repeat == 1:
            emit_input_dmas(0)
            emit_iter(0)
        else:
            assert repeat % 2 == 0, "repeat must be even (2x-unrolled loop)"
            emit_input_dmas(0)
            with tc.For_i(0, repeat // 2, 1):
                emit_iter(0, prefetch=lambda: emit_input_dmas(1))
                emit_iter(1, prefetch=lambda: emit_input_dmas(0))

    nc.compile()
    return nc


def _get_nc(n=N, repeat=1):
    key = (n, repeat)
    if key not in _compiled:
        _compiled[key] = _build(n=n, repeat=repeat)
    return _compiled[key]


def _in_maps(view1, view2, Wq, bq, Wk, bk, Wv, bv, gamma, n=N):
    b = view1.shape[0]
    f = np.ascontiguousarray
    com = {
        "wqT": f(Wq.T.astype(np.float32)),
        "wkT": f(Wk.T.astype(np.float32)),
        "wvT": f(Wv.T.astype(np.float32)),
        "bq": f(bq.reshape(1, D).astype(np.float32)),
        "bk": f(bk.reshape(1, D).astype(np.float32)),
        "bv": f(bv.reshape(1, C).astype(np.float32)),
        "gam": f(gamma.reshape(1, 1).astype(np.float32)),
    }
    in_maps = []
    for i in range(NCORES):
        bi = min(i, b - 1)  # replicate last sample if b < NCORES
        in_maps.append(
            {
                "v1": f(view1[bi].reshape(C, n).astype(np.float16)),
                "v2": f(view2[bi].reshape(C, n).astype(np.float16)),
                **com,
            }
        )
    return in_maps


def _run(nc, view1, view2, Wq, bq, Wk, bk, Wv, bv, gamma, n=N, **spmd_kwargs):
    from concourse.bass_utils import run_bass_kernel_spmd

    b = view1.shape[0]
    in_maps = _in_maps(view1, view2, Wq, bq, Wk, bk, Wv, bv, gamma, n=n)
    res = run_bass_kernel_spmd(nc, in_maps, list(range(NCORES)), **spmd_kwargs)
    outs = [res.results[i]["out"].astype(np.float32) for i in range(b)]
    return np.stack(outs, axis=0)


def kernel(view1, view2, Wq, bq, Wk, bk, Wv, bv, gamma):
    view1 = np.asarray(view1)
    b, c, h, w = view1.shape
    n = h * w
    nc = _get_nc(n=n, repeat=1)
    out = _run(
        nc,
        np.asarray(view1),
        np.asarray(view2),
        np.asarray(Wq),
        np.asarray(bq),
        np.asarray(Wk),
        np.asarray(bk),
        np.asarray(Wv),
        np.asarray(bv),
        np.asarray(gamma),
        n=n,
    )
    return out.reshape(b, c, h, w).astype(np.float32)


# revision 14
# speedup vs baseline: 1.0276x; 1.0276x over previous
"""Cross-view attention Trainium2 kernel.

Reference computation (per sample b):
    q = Wq @ x1 + bq            (D=64, N)      x1 = view1[b] as (C, N)
    k = Wk @ x2 + bk            (D, N)
    v = Wv @ x2 + bv            (C, N)
    S = q^T k                   (N, N)
    P = softmax(S, axis=-1)
    out = v @ P^T               (C, N)
    y = gamma * out + x1

Sharding: data-parallel over batch B=8 across the 8 NeuronCores (one
sample per core), no collectives.

Device algorithm (per core), measured-rate driven design:
  - Inputs arrive as f16 (host converts; f16 has 10-bit mantissa and
    |x| <= ~6 so no range issue) and stay RESIDENT in SBUF (64 KB/part
    for v1+v2), so phase 1 needs no window DMAs, no dtype-convert DVE
    passes, and the epilogue residual re-read is free.
  - q/k projections are column-packed: q occupies PE columns 0-63,
    k columns 64-127 of the same [128, nwin] PSUM tile (concurrent on
    the PE, ~605ns/pair vs 2x429ns serial). Biases ride the ACT copy
    (per-partition bias). Partition-half replicas via SBUF->SBUF DMA.
  - v projection: 4 K=128 f16 matmuls per m-tile; bias via DVE
    tensor_add against a pre-broadcast [128, C] bias tile fused with
    the PSUM->SBUF copy (the old K=1 bias matmuls cost 609ns each).
  - Attention transposed, logit chain in f16 (1 PE-cycle/row; logits
    are O(50) so f16 is safe), value chain bf16 (exp(S) spans e^+-50,
    needs bf16 range): S^T tiles (128 m-part, 512 n-free) = kT^T @ qT
    as K=64 matmuls row-packed two-at-a-time (tile_position) which run
    concurrently on the PE; exp on ScalarE (no max subtraction needed
    in bf16); PV accumulates out[c,n] += vT^T @ ex in PSUM over all m.
  - The softmax denominator l[n] = sum_m ex[m,n] runs on the Vector
    engine (running-sum tiles, f32r) + ONE K=128 ones-matmul per
    chunk, replacing 32 accl matmuls/chunk on the saturated PE.
  - Final: out = acc * (gamma/l) + v1, written f16 (host upcasts).
"""

import sys

if "/opt/trn_rl_repo" not in sys.path:
    sys.path.insert(0, "/opt/trn_rl_repo")

import numpy as np

B, C, H, W = 8, 512, 64, 64
D = C // 8            # 64
N = H * W             # 4096
CC = C // 128         # 4 chunks of the channel dim
NCORES = 8

_compiled = {}


def _build(n=N, repeat=1, nwin=512, drop=()):
    from contextlib import ExitStack

    import concourse.mybir as mybir
    import concourse.tile as tile
    from concourse import bacc

    dt = mybir.dt
    f32, f32r, bf16 = dt.float32, dt.float32r, dt.bfloat16
    f16 = dt.float16
    AF = mybir.ActivationFunctionType

    nwin = min(nwin, n)
    nch = n // nwin       # output n-chunks
    mt = n // 128         # m tiles (key/value rows per tile)

    nc = bacc.Bacc("TRN2", target_bir_lowering=False, debug=False)
    v1 = nc.dram_tensor("v1", [C, n], f16, kind="ExternalInput").ap()
    v2 = nc.dram_tensor("v2", [C, n], f16, kind="ExternalInput").ap()
    wqT = nc.dram_tensor("wqT", [C, D], f32, kind="ExternalInput").ap()
    wkT = nc.dram_tensor("wkT", [C, D], f32, kind="ExternalInput").ap()
    wvT = nc.dram_tensor("wvT", [C, C], f32, kind="ExternalInput").ap()
    bq = nc.dram_tensor("bq", [1, D], f32, kind="ExternalInput").ap()
    bk = nc.dram_tensor("bk", [1, D], f32, kind="ExternalInput").ap()
    bv = nc.dram_tensor("bv", [1, C], f32, kind="ExternalInput").ap()
    gam = nc.dram_tensor("gam", [1, 1], f32, kind="ExternalInput").ap()
    out = nc.dram_tensor("out", [C, n], f16, kind="ExternalOutput").ap()

    v1p = v1.rearrange("(cc p) n -> p cc n", p=128)
    v2p = v2.rearrange("(cc p) n -> p cc n", p=128)
    outp = out.rearrange("(cc p) n -> p cc n", p=128)

    with tile.TileContext(nc) as tc, ExitStack() as top:
        consts = top.enter_context(tc.tile_pool(name="consts", bufs=1))

        # ---- constants ----
        wq_s = consts.tile([128, CC, D], f16, tag="wq")
        wk_s = consts.tile([128, CC, D], f16, tag="wk")
        wv_s = consts.tile([128, CC, C], f16, tag="wv")
        bqc_s = consts.tile([D, 1], f32, tag="bqc")   # ACT bias column
        bkc_s = consts.tile([64, 1], f32, tag="bkc")
        bvb_s = consts.tile([128, C], f32, tag="bvb")  # bv broadcast to all parts
        gam_s = consts.tile([1, 1], f32, tag="gam")
        ones_l = consts.tile([128, 1], f32r, tag="ones_l")   # K=128,M=1 lhsT (l)
        ones_pr = consts.tile([1, 128], f32r, tag="ones_pr")  # K=1,M=128 lhsT (rb bcast)

        with ExitStack() as p0:
            wstp = p0.enter_context(tc.tile_pool(name="wst", bufs=1))
            ps0 = p0.enter_context(tc.tile_pool(name="ps0", bufs=1, space="PSUM"))
            stage_w = wstp.tile([128, CC, C], f32, tag="stage_w")
            nc.scalar.dma_start(stage_w[:, :, :D], wqT.rearrange("(cc p) d -> p cc d", p=128))
            nc.vector.tensor_copy(wq_s[:], stage_w[:, :, :D])
            nc.scalar.dma_start(stage_w[:, :, D : 2 * D], wkT.rearrange("(cc p) d -> p cc d", p=128))
            nc.vector.tensor_copy(wk_s[:], stage_w[:, :, D : 2 * D])
            nc.scalar.dma_start(stage_w[:], wvT.rearrange("(cc p) c -> p cc c", p=128))
            nc.vector.tensor_copy(wv_s[:], stage_w[:])

            nc.scalar.dma_start(bqc_s[:], bq.rearrange("o d -> d o"))
            nc.scalar.dma_start(bkc_s[:], bk.rearrange("o d -> d o"))
            nc.scalar.dma_start(gam_s[:], gam[:])

            ones_f32 = wstp.tile([128, 128], f32, tag="ones_f32")
            nc.vector.memset(ones_f32[:], 1.0)
            nc.vector.tensor_copy(ones_l[:], ones_f32[:, :1])
            nc.vector.tensor_copy(ones_pr[:], ones_f32[:1, :])

            # bv broadcast: [1,C] -> [128,C] via K=1 ones matmul
            stage_b = wstp.tile([1, C], f32, tag="stage_b")
            nc.scalar.dma_start(stage_b[:], bv[:])
            ones_c = wstp.tile([1, 128], f32, tag="ones_c")
            nc.vector.tensor_copy(ones_c[:], ones_f32[:1, :])
            bvb_ps = ps0.tile([128, C], f32, tag="bvb_ps")
            nc.tensor.matmul(bvb_ps[:], ones_c[:], stage_b[:], start=True, stop=True)
            nc.vector.tensor_copy(bvb_s[:], bvb_ps[:])

        per = top.enter_context(tc.tile_pool(name="persist", bufs=1))
        v2_s = per.tile([128, CC, n], f16, tag="v2s")
        # qT/kT duplicated across both partition halves for the
        # row-packed (tile_position) S^T pairs
        qT_s = per.tile([128, n], f16, tag="qT")
        kT_s = per.tile([128, n], f16, tag="kT")
        vT_s = per.tile([128, mt, C], f16, tag="vT")

        def emit_input_dmas(vi):
            # v2 only (v1 is streamed in windows by the q-projection and
            # the epilogue); called mid-phase-2 as a prefetch so the next
            # iteration's vproj never waits on DMA and the PE never idles
            # long enough for the HAM clock gate to re-throttle it
            nc.sync.dma_start(v2_s[:, :1, :], v2p[:, :1, :])
            nc.gpsimd.dma_start(v2_s[:, 1:2, :], v2p[:, 1:2, :])
            nc.scalar.dma_start(v2_s[:, 2:3, :], v2p[:, 2:3, :])
            nc.sync.dma_start(v2_s[:, 3:, :], v2p[:, 3:, :])

        def emit_iter(rep, prefetch=None):
            with ExitStack() as rctx:
                # ================= phase 1: projections =================
                if "proj" in drop:
                    nc.vector.memset(qT_s[:], 0.01)
                    nc.vector.memset(kT_s[:], 0.01)
                    nc.vector.memset(vT_s[:], 0.01)
                with ExitStack() as p1:
                    nch1 = 0 if "proj" in drop else nch
                    psqk = p1.enter_context(
                        tc.tile_pool(name=f"psqk{rep}", bufs=2, space="PSUM")
                    )
                    psv = p1.enter_context(
                        tc.tile_pool(name=f"psv{rep}", bufs=2, space="PSUM")
                    )
                    v1wp = p1.enter_context(tc.tile_pool(name=f"v1w{rep}", bufs=3))

                    # v projection first: gated only on the v2 DMA, so the
                    # PE starts ~10us in; the v1 DMA hides behind it
                    for m in range(0 if "proj" in drop else mt):
                        miw = slice(m * 128, (m + 1) * 128)
                        pv = psv.tile([128, C], f32, tag="psv")
                        for cc in range(CC):
                            nc.tensor.matmul(
                                pv[:],
                                v2_s[:, cc, miw],
                                wv_s[:, cc, :],
                                start=(cc == 0),
                                stop=(cc == CC - 1),
                            )
                        nc.vector.tensor_add(vT_s[:, m, :], pv[:], bvb_s[:])

                    for j in range(nch1):
                        jw = slice(j * nwin, (j + 1) * nwin)
                        v1w = v1wp.tile([128, CC, nwin], f16, tag="v1w", name="v1w")
                        eng = (nc.sync, nc.gpsimd, nc.scalar)[j % 3]
                        eng.dma_start(v1w[:], v1p[:, :, jw])
                        # q in PE columns 0-63, k in columns 64-127:
                        # concurrent on disjoint PE column groups; the two
                        # accumulation groups touch disjoint partition
                        # halves of the bank, so interleaved start is safe
                        ps = psqk.tile([128, nwin], f32, tag="psqk")
                        for cc in range(CC):
                            nc.tensor.matmul(
                                ps[:64, :],
                                wq_s[:, cc, :],
                                v1w[:, cc, :],
                                start=(cc == 0),
                                stop=(cc == CC - 1),
                                tile_position=(0, 0),
                            )
                            nc.tensor.matmul(
                                ps[64:128, :],
                                wk_s[:, cc, :],
                                v2_s[:, cc, jw],
                                start=(cc == 0),
                                stop=(cc == CC - 1),
                                tile_position=(0, 64),
                                skip_group_check=True,
                            )
                        nc.scalar.activation(
                            qT_s[:64, jw], ps[:64, :], AF.Identity, bias=bqc_s[:]
                        )
                        nc.scalar.activation(
                            kT_s[64:128, jw], ps[64:128, :], AF.Identity, bias=bkc_s[:]
                        )
                        nc.sync.dma_start(qT_s[64:128, jw], qT_s[:64, jw])
                        nc.gpsimd.dma_start(kT_s[:64, jw], kT_s[64:128, jw])

                # ================= phase 2: attention =================
                with ExitStack() as p2:
                    psS = p2.enter_context(
                        tc.tile_pool(name=f"psS{rep}", bufs=3, space="PSUM")
                    )
                    psA = p2.enter_context(
                        tc.tile_pool(name=f"psA{rep}", bufs=1, space="PSUM")
                    )
                    psL = p2.enter_context(
                        tc.tile_pool(name=f"psL{rep}", bufs=1, space="PSUM")
                    )
                    expp = p2.enter_context(tc.tile_pool(name=f"expp{rep}", bufs=14))
                    parep = p2.enter_context(tc.tile_pool(name=f"parep{rep}", bufs=3))
                    sump = p2.enter_context(tc.tile_pool(name=f"sump{rep}", bufs=2))
                    smalls = p2.enter_context(tc.tile_pool(name=f"smalls{rep}", bufs=2))
                    rbp = p2.enter_context(tc.tile_pool(name=f"rbp{rep}", bufs=2))
                    v1cp = p2.enter_context(tc.tile_pool(name=f"v1cp{rep}", bufs=6))
                    outp_sb = p2.enter_context(tc.tile_pool(name=f"outp{rep}", bufs=3))

                    if "pv" in drop or "accl" in drop:
                        inip = p2.enter_context(tc.tile_pool(name=f"inip{rep}", bufs=1))
                        ini = inip.tile([128, nwin], f32, tag="ini")
                        nc.vector.memset(ini[:], 1.0)
                    if "exp" in drop:
                        exst = p2.enter_context(tc.tile_pool(name=f"exst{rep}", bufs=1))
                        ex_static = [
                            exst.tile([128, nwin], bf16, tag=f"exs{h}", name=f"exs{h}")
                            for h in (0, 1)
                        ]
                        nc.vector.memset(ex_static[0][:], 0.01)
                        nc.vector.memset(ex_static[1][:], 0.01)

                    def emit_epilogue(j, accs, accl):
                        # y = acc * (gamma/l) + view1
                        jw = slice(j * nwin, (j + 1) * nwin)
                        v1cs = []
                        for ct in range(CC):
                            v1c = v1cp.tile([128, nwin], f16, tag="v1c", name="v1c")
                            eng = (nc.sync, nc.gpsimd, nc.scalar)[ct % 3]
                            eng.dma_start(v1c[:], v1p[:, ct, jw])
                            v1cs.append(v1c)
                        l_sb = smalls.tile([1, nwin], f32, tag="l", name="l_sb")
                        nc.vector.tensor_copy(l_sb[:], accl[:])
                        r_sb = smalls.tile([1, nwin], f32, tag="r", name="r_sb")
                        nc.vector.reciprocal(r_sb[:], l_sb[:])
                        rg_sb = smalls.tile([1, nwin], f32r, tag="rg", name="rg_sb")
                        nc.scalar.activation(rg_sb[:], r_sb[:], AF.Copy, scale=gam_s[:])
                        rb_ps = psL.tile([128, nwin], f32, tag="accl", name="rb_ps")
                        nc.tensor.matmul(rb_ps[:], ones_pr[:], rg_sb[:], start=True, stop=True)
                        rb_sb = rbp.tile([128, nwin], f32, tag="rb", name="rb_sb")
                        nc.vector.tensor_copy(rb_sb[:], rb_ps[:])
                        for ct in range(CC):
                            t_sb = outp_sb.tile([128, nwin], f32, tag="t", name="t_sb")
                            nc.vector.tensor_mul(t_sb[:], accs[ct][:], rb_sb[:])
                            o_sb = outp_sb.tile([128, nwin], f16, tag="o", name="o_sb")
                            nc.vector.tensor_add(o_sb[:], t_sb[:], v1cs[ct][:])
                            nc.sync.dma_start(outp[:, ct, jw], o_sb[:])

                    npairs = mt // 2
                    pend_epi = None
                    for j in range(nch):
                        jw = slice(j * nwin, (j + 1) * nwin)
                        # one PSUM tile (= one full bank) per output c-chunk:
                        # accumulation groups must not share a bank
                        accs = [
                            psA.tile([128, nwin], f32, tag=f"acc{ct}", name=f"acc{ct}")
                            for ct in range(CC)
                        ]
                        if "pv" in drop:
                            for ct in range(CC):
                                nc.vector.tensor_copy(accs[ct][:], ini[:])
                        # software pipeline: issue S^T/exp of pair i+1 before
                        # the P.V matmuls of pair i, so ScalarE's exp overlaps
                        # TensorE's P.V; the previous chunk's epilogue is
                        # emitted after this chunk's first S^T pair
                        prev_exs = None
                        sumE = None
                        pares = []
                        for m2 in range(npairs + 1):
                            exs = []
                            if m2 < npairs:
                                sts = []
                                for half in (0, 1):
                                    m = 2 * m2 + half
                                    mw = slice(m * 128, (m + 1) * 128)
                                    hp = slice(64 * half, 64 * half + 64)
                                    if "st" in drop:
                                        sts.append(None)
                                        continue
                                    st = psS.tile([128, nwin], f32, tag="st", name="st")
                                    nc.tensor.matmul(
                                        st[:],
                                        kT_s[hp, mw],
                                        qT_s[hp, jw],
                                        start=True,
                                        stop=True,
                                        tile_position=(64 * half, 0),
                                    )
                                    sts.append(st)
                                if "exp" in drop:
                                    exs = ex_static
                                else:
                                    for half in (0, 1):
                                        ex = expp.tile(
                                            [128, nwin], bf16, tag="ex", name="ex"
                                        )
                                        nc.scalar.activation(
                                            ex[:], sts[half][:], AF.Exp
                                        )
                                        exs.append(ex)
                            if m2 > 0:
                                for half in (0, 1):
                                    m = 2 * (m2 - 1) + half
                                    ex = prev_exs[half]
                                    for ct in range(CC if "pv" not in drop else 0):
                                        nc.tensor.matmul(
                                            accs[ct][:],
                                            vT_s[:, m, ct * 128 : (ct + 1) * 128],
                                            ex[:],
                                            start=(m == 0),
                                            stop=(m == mt - 1),
                                        )
                                if "accl" not in drop:
                                    # pair-partial on DVE: frees the ex tiles
                                    # immediately (the serial denominator
                                    # chain below runs on these, so the exp
                                    # pool never backs up behind it)
                                    pare = parep.tile(
                                        [128, nwin], bf16, tag="pare", name="pare"
                                    )
                                    nc.vector.tensor_add(
                                        pare[:], prev_exs[0][:], prev_exs[1][:]
                                    )
                                    if sumE is None:
                                        sumE = pare
                                    else:
                                        sacc = sump.tile(
                                            [128, nwin], f32r, tag="sumE", name="sacc"
                                        )
                                        nc.vector.tensor_add(
                                            sacc[:], sumE[:], pare[:]
                                        )
                                        sumE = sacc
                            if m2 == 1 and pend_epi is not None:
                                # emitted AFTER this chunk's first PV block:
                                # the rb broadcast matmul inside depends on a
                                # DVE->ACT chain, and the in-order PE queue
                                # must have PV work ahead of it to hide that
                                emit_epilogue(*pend_epi)
                                pend_epi = None
                                if j == 1 and prefetch is not None:
                                    prefetch()
                            prev_exs = exs
                        accl = psL.tile([1, nwin], f32, tag="accl")
                        if "accl" not in drop:
                            nc.tensor.matmul(
                                accl[:], ones_l[:], sumE[:], start=True, stop=True
                            )
                        else:
                            nc.vector.tensor_copy(accl[:], ini[:1, :])
                        pend_epi = (j, accs, accl)
                    emit_epilogue(*pend_epi)

        if repeat == 1:
            emit_input_dmas(0)
            emit_iter(0)
        else:
            assert repeat % 2 == 0, "repeat must be even (2x-unrolled loop)"
            emit_input_dmas(0)
            with tc.For_i(0, repeat // 2, 1):
                emit_iter(0, prefetch=lambda: emit_input_dmas(1))
                emit_iter(1, prefetch=lambda: emit_input_dmas(0))

    nc.compile()
    return nc


def _get_nc(n=N, repeat=1):
    key = (n, repeat)
    if key not in _compiled:
        _compiled[key] = _build(n=n, repeat=repeat)
    return _compiled[key]


def _in_maps(view1, view2, Wq, bq, Wk, bk, Wv, bv, gamma, n=N):
    b = view1.shape[0]
    f = np.ascontiguousarray
    com = {
        "wqT": f(Wq.T.astype(np.float32)),
        "wkT": f(Wk.T.astype(np.float32)),
        "wvT": f(Wv.T.astype(np.float32)),
        "bq": f(bq.reshape(1, D).astype(np.float32)),
        "bk": f(bk.reshape(1, D).astype(np.float32)),
        "bv": f(bv.reshape(1, C).astype(np.float32)),
        "gam": f(gamma.reshape(1, 1).astype(np.float32)),
    }
    in_maps = []
    for i in range(NCORES):
        bi = min(i, b - 1)  # replicate last sample if b < NCORES
        in_maps.append(
            {
                "v1": f(view1[bi].reshape(C, n).astype(np.float16)),
                "v2": f(view2[bi].reshape(C, n).astype(np.float16)),
                **com,
            }
        )
    return in_maps


def _run(nc, view1, view2, Wq, bq, Wk, bk, Wv, bv, gamma, n=N, **spmd_kwargs):
    from concourse.bass_utils import run_bass_kernel_spmd

    b = view1.shape[0]
    in_maps = _in_maps(view1, view2, Wq, bq, Wk, bk, Wv, bv, gamma, n=n)
    res = run_bass_kernel_spmd(nc, in_maps, list(range(NCORES)), **spmd_kwargs)
    outs = [res.results[i]["out"].astype(np.float32) for i in range(b)]
    return np.stack(outs, axis=0)


def kernel(view1, view2, Wq, bq, Wk, bk, Wv, bv, gamma):
    view1 = np.asarray(view1)
    b, c, h, w = view1.shape
    n = h * w
    nc = _get_nc(n=n, repeat=1)
    out = _run(
        nc,
        np.asarray(view1),
        np.asarray(view2),
        np.asarray(Wq),
        np.asarray(bq),
        np.asarray(Wk),
        np.asarray(bk),
        np.asarray(Wv),
        np.asarray(bv),
        np.asarray(gamma),
        n=n,
    )
    return out.reshape(b, c, h, w).astype(np.float32)


# revision 15
# speedup vs baseline: 1.0937x; 1.0643x over previous
"""Cross-view attention Trainium2 kernel.

Reference computation (per sample b):
    q = Wq @ x1 + bq            (D=64, N)      x1 = view1[b] as (C, N)
    k = Wk @ x2 + bk            (D, N)
    v = Wv @ x2 + bv            (C, N)
    S = q^T k                   (N, N)
    P = softmax(S, axis=-1)
    out = v @ P^T               (C, N)
    y = gamma * out + x1

Sharding: data-parallel over batch B=8 across the 8 NeuronCores (one
sample per core), no collectives.

Device algorithm (per core), measured-rate driven design:
  - Inputs arrive as f16 (host converts; f16 has 10-bit mantissa and
    |x| <= ~6 so no range issue) and stay RESIDENT in SBUF (64 KB/part
    for v1+v2), so phase 1 needs no window DMAs, no dtype-convert DVE
    passes, and the epilogue residual re-read is free.
  - q/k projections are column-packed: q occupies PE columns 0-63,
    k columns 64-127 of the same [128, nwin] PSUM tile (concurrent on
    the PE, ~605ns/pair vs 2x429ns serial). Biases ride the ACT copy
    (per-partition bias). Partition-half replicas via SBUF->SBUF DMA.
  - v projection: 4 K=128 f16 matmuls per m-tile; bias via DVE
    tensor_add against a pre-broadcast [128, C] bias tile fused with
    the PSUM->SBUF copy (the old K=1 bias matmuls cost 609ns each).
  - Attention transposed, logit chain in f16 (1 PE-cycle/row; logits
    are O(50) so f16 is safe), value chain bf16 (exp(S) spans e^+-50,
    needs bf16 range): S^T tiles (128 m-part, 512 n-free) = kT^T @ qT
    as K=64 matmuls row-packed two-at-a-time (tile_position) which run
    concurrently on the PE; exp on ScalarE (no max subtraction needed
    in bf16); PV accumulates out[c,n] += vT^T @ ex in PSUM over all m.
  - The softmax denominator l[n] = sum_m ex[m,n] runs on the Vector
    engine (running-sum tiles, f32r) + ONE K=128 ones-matmul per
    chunk, replacing 32 accl matmuls/chunk on the saturated PE.
  - Final: out = acc * (gamma/l) + v1, written f16 (host upcasts).
"""

import sys

if "/opt/trn_rl_repo" not in sys.path:
    sys.path.insert(0, "/opt/trn_rl_repo")

import numpy as np

B, C, H, W = 8, 512, 64, 64
D = C // 8            # 64
N = H * W             # 4096
CC = C // 128         # 4 chunks of the channel dim
NCORES = 8

_compiled = {}


def _build(n=N, repeat=1, nwin=512, drop=()):
    from contextlib import ExitStack

    import concourse.mybir as mybir
    import concourse.tile as tile
    from concourse import bacc

    dt = mybir.dt
    f32, f32r, bf16 = dt.float32, dt.float32r, dt.bfloat16
    f16 = dt.float16
    AF = mybir.ActivationFunctionType

    nwin = min(nwin, n)
    nch = n // nwin       # output n-chunks
    mt = n // 128         # m tiles (key/value rows per tile)

    nc = bacc.Bacc("TRN2", target_bir_lowering=False, debug=False)
    v1 = nc.dram_tensor("v1", [C, n], f16, kind="ExternalInput").ap()
    v2 = nc.dram_tensor("v2", [C, n], f16, kind="ExternalInput").ap()
    wqT = nc.dram_tensor("wqT", [C, D], f32, kind="ExternalInput").ap()
    wkT = nc.dram_tensor("wkT", [C, D], f32, kind="ExternalInput").ap()
    wvT = nc.dram_tensor("wvT", [C, C], f32, kind="ExternalInput").ap()
    bq = nc.dram_tensor("bq", [1, D], f32, kind="ExternalInput").ap()
    bk = nc.dram_tensor("bk", [1, D], f32, kind="ExternalInput").ap()
    bv = nc.dram_tensor("bv", [1, C], f32, kind="ExternalInput").ap()
    gam = nc.dram_tensor("gam", [1, 1], f32, kind="ExternalInput").ap()
    out = nc.dram_tensor("out", [C, n], f16, kind="ExternalOutput").ap()

    v1p = v1.rearrange("(cc p) n -> p cc n", p=128)
    v2p = v2.rearrange("(cc p) n -> p cc n", p=128)
    outp = out.rearrange("(cc p) n -> p cc n", p=128)

    with tile.TileContext(nc) as tc, ExitStack() as top:
        consts = top.enter_context(tc.tile_pool(name="consts", bufs=1))

        # ---- constants ----
        wq_s = consts.tile([128, CC, D], f16, tag="wq")
        wk_s = consts.tile([128, CC, D], f16, tag="wk")
        wv_s = consts.tile([128, CC, C], f16, tag="wv")
        bqc_s = consts.tile([D, 1], f32, tag="bqc")   # ACT bias column
        bkc_s = consts.tile([64, 1], f32, tag="bkc")
        bvb_s = consts.tile([128, C], f32, tag="bvb")  # bv broadcast to all parts
        gam_s = consts.tile([1, 1], f32, tag="gam")
        ones_l = consts.tile([128, 1], f32r, tag="ones_l")   # K=128,M=1 lhsT (l)
        ones_pr = consts.tile([1, 128], f32r, tag="ones_pr")  # K=1,M=128 lhsT (rb bcast)

        with ExitStack() as p0:
            wstp = p0.enter_context(tc.tile_pool(name="wst", bufs=1))
            ps0 = p0.enter_context(tc.tile_pool(name="ps0", bufs=1, space="PSUM"))
            stage_w = wstp.tile([128, CC, C], f32, tag="stage_w")
            nc.scalar.dma_start(stage_w[:, :, :D], wqT.rearrange("(cc p) d -> p cc d", p=128))
            nc.vector.tensor_copy(wq_s[:], stage_w[:, :, :D])
            nc.scalar.dma_start(stage_w[:, :, D : 2 * D], wkT.rearrange("(cc p) d -> p cc d", p=128))
            nc.vector.tensor_copy(wk_s[:], stage_w[:, :, D : 2 * D])
            nc.scalar.dma_start(stage_w[:], wvT.rearrange("(cc p) c -> p cc c", p=128))
            nc.vector.tensor_copy(wv_s[:], stage_w[:])

            nc.scalar.dma_start(bqc_s[:], bq.rearrange("o d -> d o"))
            nc.scalar.dma_start(bkc_s[:], bk.rearrange("o d -> d o"))
            nc.scalar.dma_start(gam_s[:], gam[:])

            ones_f32 = wstp.tile([128, 128], f32, tag="ones_f32")
            nc.vector.memset(ones_f32[:], 1.0)
            nc.vector.tensor_copy(ones_l[:], ones_f32[:, :1])
            nc.vector.tensor_copy(ones_pr[:], ones_f32[:1, :])

            # bv broadcast: [1,C] -> [128,C] via K=1 ones matmul
            stage_b = wstp.tile([1, C], f32, tag="stage_b")
            nc.scalar.dma_start(stage_b[:], bv[:])
            ones_c = wstp.tile([1, 128], f32, tag="ones_c")
            nc.vector.tensor_copy(ones_c[:], ones_f32[:1, :])
            bvb_ps = ps0.tile([128, C], f32, tag="bvb_ps")
            nc.tensor.matmul(bvb_ps[:], ones_c[:], stage_b[:], start=True, stop=True)
            nc.vector.tensor_copy(bvb_s[:], bvb_ps[:])

        per = top.enter_context(tc.tile_pool(name="persist", bufs=1))
        v1b = [per.tile([128, CC, n], f16, tag=f"v1s{i}", name=f"v1s{i}") for i in (0, 1)]
        v2_s = per.tile([128, CC, n], f16, tag="v2s")
        # qT/kT duplicated across both partition halves for the
        # row-packed (tile_position) S^T pairs
        qT_s = per.tile([128, n], f16, tag="qT")
        kT_s = per.tile([128, n], f16, tag="kT")
        vT_s = per.tile([128, mt, C], f16, tag="vT")

        def emit_input_dmas(vi):
            # v2 first (gates vproj), v1 into buffer vi behind it; called
            # mid-phase-2 as a prefetch so the next iteration's vproj
            # never waits on DMA and the PE never idles long enough for
            # the HAM clock gate to re-throttle it
            v1_s = v1b[vi]
            nc.sync.dma_start(v2_s[:, :1, :], v2p[:, :1, :])
            nc.gpsimd.dma_start(v2_s[:, 1:2, :], v2p[:, 1:2, :])
            nc.scalar.dma_start(v2_s[:, 2:3, :], v2p[:, 2:3, :])
            nc.sync.dma_start(v2_s[:, 3:, :], v2p[:, 3:, :])
            nc.gpsimd.dma_start(v1_s[:, :2, :], v1p[:, :2, :])
            nc.scalar.dma_start(v1_s[:, 2:, :], v1p[:, 2:, :])

        def emit_iter(rep, prefetch=None):
            v1_s = v1b[rep % 2]
            with ExitStack() as rctx:
                # ================= phase 1: projections =================
                if "proj" in drop:
                    nc.vector.memset(qT_s[:], 0.01)
                    nc.vector.memset(kT_s[:], 0.01)
                    nc.vector.memset(vT_s[:], 0.01)
                with ExitStack() as p1:
                    nch1 = 0 if "proj" in drop else nch
                    psqk = p1.enter_context(
                        tc.tile_pool(name=f"psqk{rep}", bufs=2, space="PSUM")
                    )
                    psv = p1.enter_context(
                        tc.tile_pool(name=f"psv{rep}", bufs=2, space="PSUM")
                    )

                    # v projection first: gated only on the v2 DMA, so the
                    # PE starts ~10us in; the v1 DMA hides behind it
                    for m in range(0 if "proj" in drop else mt):
                        miw = slice(m * 128, (m + 1) * 128)
                        pv = psv.tile([128, C], f32, tag="psv")
                        for cc in range(CC):
                            nc.tensor.matmul(
                                pv[:],
                                v2_s[:, cc, miw],
                                wv_s[:, cc, :],
                                start=(cc == 0),
                                stop=(cc == CC - 1),
                            )
                        nc.vector.tensor_add(vT_s[:, m, :], pv[:], bvb_s[:])

                    for j in range(nch1):
                        jw = slice(j * nwin, (j + 1) * nwin)
                        # q in PE columns 0-63, k in columns 64-127:
                        # concurrent on disjoint PE column groups; the two
                        # accumulation groups touch disjoint partition
                        # halves of the bank, so interleaved start is safe
                        ps = psqk.tile([128, nwin], f32, tag="psqk")
                        for cc in range(CC):
                            nc.tensor.matmul(
                                ps[:64, :],
                                wq_s[:, cc, :],
                                v1_s[:, cc, jw],
                                start=(cc == 0),
                                stop=(cc == CC - 1),
                                tile_position=(0, 0),
                            )
                            nc.tensor.matmul(
                                ps[64:128, :],
                                wk_s[:, cc, :],
                                v2_s[:, cc, jw],
                                start=(cc == 0),
                                stop=(cc == CC - 1),
                                tile_position=(0, 64),
                                skip_group_check=True,
                            )
                        nc.scalar.activation(
                            qT_s[:64, jw], ps[:64, :], AF.Identity, bias=bqc_s[:]
                        )
                        nc.scalar.activation(
                            kT_s[64:128, jw], ps[64:128, :], AF.Identity, bias=bkc_s[:]
                        )
                        nc.sync.dma_start(qT_s[64:128, jw], qT_s[:64, jw])
                        nc.gpsimd.dma_start(kT_s[:64, jw], kT_s[64:128, jw])

                # ================= phase 2: attention =================
                with ExitStack() as p2:
                    psS = p2.enter_context(
                        tc.tile_pool(name=f"psS{rep}", bufs=3, space="PSUM")
                    )
                    psA = p2.enter_context(
                        tc.tile_pool(name=f"psA{rep}", bufs=1, space="PSUM")
                    )
                    psL = p2.enter_context(
                        tc.tile_pool(name=f"psL{rep}", bufs=1, space="PSUM")
                    )
                    expp = p2.enter_context(tc.tile_pool(name=f"expp{rep}", bufs=14))
                    parep = p2.enter_context(tc.tile_pool(name=f"parep{rep}", bufs=3))
                    sump = p2.enter_context(tc.tile_pool(name=f"sump{rep}", bufs=2))
                    smalls = p2.enter_context(tc.tile_pool(name=f"smalls{rep}", bufs=2))
                    rbp = p2.enter_context(tc.tile_pool(name=f"rbp{rep}", bufs=2))
                    outp_sb = p2.enter_context(tc.tile_pool(name=f"outp{rep}", bufs=3))

                    if "pv" in drop or "accl" in drop:
                        inip = p2.enter_context(tc.tile_pool(name=f"inip{rep}", bufs=1))
                        ini = inip.tile([128, nwin], f32, tag="ini")
                        nc.vector.memset(ini[:], 1.0)
                    if "exp" in drop:
                        exst = p2.enter_context(tc.tile_pool(name=f"exst{rep}", bufs=1))
                        ex_static = [
                            exst.tile([128, nwin], bf16, tag=f"exs{h}", name=f"exs{h}")
                            for h in (0, 1)
                        ]
                        nc.vector.memset(ex_static[0][:], 0.01)
                        nc.vector.memset(ex_static[1][:], 0.01)

                    def emit_epilogue(j, accs, accl):
                        # y = acc * (gamma/l) + view1
                        jw = slice(j * nwin, (j + 1) * nwin)

                        l_sb = smalls.tile([1, nwin], f32, tag="l", name="l_sb")
                        nc.vector.tensor_copy(l_sb[:], accl[:])
                        r_sb = smalls.tile([1, nwin], f32, tag="r", name="r_sb")
                        nc.vector.reciprocal(r_sb[:], l_sb[:])
                        rg_sb = smalls.tile([1, nwin], f32r, tag="rg", name="rg_sb")
                        nc.scalar.activation(rg_sb[:], r_sb[:], AF.Copy, scale=gam_s[:])
                        rb_ps = psL.tile([128, nwin], f32, tag="accl", name="rb_ps")
                        nc.tensor.matmul(rb_ps[:], ones_pr[:], rg_sb[:], start=True, stop=True)
                        rb_sb = rbp.tile([128, nwin], f32, tag="rb", name="rb_sb")
                        nc.vector.tensor_copy(rb_sb[:], rb_ps[:])
                        for ct in range(CC):
                            t_sb = outp_sb.tile([128, nwin], f32, tag="t", name="t_sb")
                            nc.vector.tensor_mul(t_sb[:], accs[ct][:], rb_sb[:])
                            o_sb = outp_sb.tile([128, nwin], f16, tag="o", name="o_sb")
                            nc.vector.tensor_add(o_sb[:], t_sb[:], v1_s[:, ct, jw])
                            nc.sync.dma_start(outp[:, ct, jw], o_sb[:])

                    npairs = mt // 2
                    pend_epi = None
                    for j in range(nch):
                        jw = slice(j * nwin, (j + 1) * nwin)
                        # one PSUM tile (= one full bank) per output c-chunk:
                        # accumulation groups must not share a bank
                        accs = [
                            psA.tile([128, nwin], f32, tag=f"acc{ct}", name=f"acc{ct}")
                            for ct in range(CC)
                        ]
                        if "pv" in drop:
                            for ct in range(CC):
                                nc.vector.tensor_copy(accs[ct][:], ini[:])
                        # software pipeline: issue S^T/exp of pair i+1 before
                        # the P.V matmuls of pair i, so ScalarE's exp overlaps
                        # TensorE's P.V; the previous chunk's epilogue is
                        # emitted after this chunk's first S^T pair
                        prev_exs = None
                        sumE = None
                        pares = []
                        for m2 in range(npairs + 1):
                            exs = []
                            if m2 < npairs:
                                sts = []
                                for half in (0, 1):
                                    m = 2 * m2 + half
                                    mw = slice(m * 128, (m + 1) * 128)
                                    hp = slice(64 * half, 64 * half + 64)
                                    if "st" in drop:
                                        sts.append(None)
                                        continue
                                    st = psS.tile([128, nwin], f32, tag="st", name="st")
                                    nc.tensor.matmul(
                                        st[:],
                                        kT_s[hp, mw],
                                        qT_s[hp, jw],
                                        start=True,
                                        stop=True,
                                        tile_position=(64 * half, 0),
                                    )
                                    sts.append(st)
                                if "exp" in drop:
                                    exs = ex_static
                                else:
                                    for half in (0, 1):
                                        ex = expp.tile(
                                            [128, nwin], bf16, tag="ex", name="ex"
                                        )
                                        nc.scalar.activation(
                                            ex[:], sts[half][:], AF.Exp
                                        )
                                        exs.append(ex)
                            if m2 > 0:
                                for half in (0, 1):
                                    m = 2 * (m2 - 1) + half
                                    ex = prev_exs[half]
                                    for ct in range(CC if "pv" not in drop else 0):
                                        nc.tensor.matmul(
                                            accs[ct][:],
                                            vT_s[:, m, ct * 128 : (ct + 1) * 128],
                                            ex[:],
                                            start=(m == 0),
                                            stop=(m == mt - 1),
                                        )
                                if "accl" not in drop:
                                    # pair-partial on DVE: frees the ex tiles
                                    # immediately (the serial denominator
                                    # chain below runs on these, so the exp
                                    # pool never backs up behind it)
                                    pare = parep.tile(
                                        [128, nwin], bf16, tag="pare", name="pare"
                                    )
                                    nc.vector.tensor_add(
                                        pare[:], prev_exs[0][:], prev_exs[1][:]
                                    )
                                    if sumE is None:
                                        sumE = pare
                                    else:
                                        sacc = sump.tile(
                                            [128, nwin], f32r, tag="sumE", name="sacc"
                                        )
                                        nc.vector.tensor_add(
                                            sacc[:], sumE[:], pare[:]
                                        )
                                        sumE = sacc
                            if m2 == 1 and pend_epi is not None:
                                # emitted AFTER this chunk's first PV block:
                                # the rb broadcast matmul inside depends on a
                                # DVE->ACT chain, and the in-order PE queue
                                # must have PV work ahead of it to hide that
                                emit_epilogue(*pend_epi)
                                pend_epi = None
                                if j == 1 and prefetch is not None:
                                    prefetch()
                            prev_exs = exs
                        accl = psL.tile([1, nwin], f32, tag="accl")
                        if "accl" not in drop:
                            nc.tensor.matmul(
                                accl[:], ones_l[:], sumE[:], start=True, stop=True
                            )
                        else:
                            nc.vector.tensor_copy(accl[:], ini[:1, :])
                        pend_epi = (j, accs, accl)
                    emit_epilogue(*pend_epi)

        if repeat == 1:
            emit_input_dmas(0)
            emit_iter(0)
        else:
            assert repeat % 2 == 0, "repeat must be even (2x-unrolled loop)"
            emit_input_dmas(0)
            with tc.For_i(0, repeat // 2, 1):
                emit_iter(0, prefetch=lambda: emit_input_dmas(1))
                emit_iter(1, prefetch=lambda: emit_input_dmas(0))

    nc.compile()
    return nc


def _get_nc(n=N, repeat=1):
    key = (n, repeat)
    if key not in _compiled:
        _compiled[key] = _build(n=n, repeat=repeat)
    return _compiled[key]


def _in_maps(view1, view2, Wq, bq, Wk, bk, Wv, bv, gamma, n=N):
    b = view1.shape[0]
    f = np.ascontiguousarray
    com = {
        "wqT": f(Wq.T.astype(np.float32)),
        "wkT": f(Wk.T.astype(np.float32)),
        "wvT": f(Wv.T.astype(np.float32)),
        "bq": f(bq.reshape(1, D).astype(np.float32)),
        "bk": f(bk.reshape(1, D).astype(np.float32)),
        "bv": f(bv.reshape(1, C).astype(np.float32)),
        "gam": f(gamma.reshape(1, 1).astype(np.float32)),
    }
    in_maps = []
    for i in range(NCORES):
        bi = min(i, b - 1)  # replicate last sample if b < NCORES
        in_maps.append(
            {
                "v1": f(view1[bi].reshape(C, n).astype(np.float16)),
                "v2": f(view2[bi].reshape(C, n).astype(np.float16)),
                **com,
            }
        )
    return in_maps


def _run(nc, view1, view2, Wq, bq, Wk, bk, Wv, bv, gamma, n=N, **spmd_kwargs):
    from concourse.bass_utils import run_bass_kernel_spmd

    b = view1.shape[0]
    in_maps = _in_maps(view1, view2, Wq, bq, Wk, bk, Wv, bv, gamma, n=n)
    res = run_bass_kernel_spmd(nc, in_maps, list(range(NCORES)), **spmd_kwargs)
    outs = [res.results[i]["out"].astype(np.float32) for i in range(b)]
    return np.stack(outs, axis=0)


def kernel(view1, view2, Wq, bq, Wk, bk, Wv, bv, gamma):
    view1 = np.asarray(view1)
    b, c, h, w = view1.shape
    n = h * w
    nc = _get_nc(n=n, repeat=1)
    out = _run(
        nc,
        np.asarray(view1),
        np.asarray(view2),
        np.asarray(Wq),
        np.asarray(bq),
        np.asarray(Wk),
        np.asarray(bk),
        np.asarray(Wv),
        np.asarray(bv),
        np.asarray(gamma),
        n=n,
    )
    return out.reshape(b, c, h, w).astype(np.float32)
